# revision 8
# baseline (speedup 1.0000x reference)
"""DecodeDetections keypoint-decode kernel for Trainium2 (8 NeuronCores).

Computation per box (original 20 input channels -> 12 output channels):
  out[0:2]    = in[0:2]                                       (class scores)
  out[2+2k]   = (in[2+2k] * in[16] * in[14] + in[12]) * 512   k=0..4  (kp x)
  out[3+2k]   = (in[3+2k] * in[17] * in[15] + in[13]) * 512   k=0..4  (kp y)
Channels 18,19 are unused; out[0:2] is an exact passthrough of in[0:2].

The kernel is DMA-bound (16 DMA engines/core @ ~25.6 GB/s each), so the
device moves only the bytes that feed real math, in fp16:
  - host slices channels 2:18 (16ch) and converts to fp16  -> device input
  - device computes the 10 kp channels in fp16             -> device output
  - host copies class channels from the f32 input directly (exact) and
    upcasts kp to f32.  Output absmax ~17k fits fp16 range; rel-err ~4e-4.

Device-side layout: batch axis split 4-per-core; rows tiled partition-major
(tile t, partition p holds rows [t*128*J + p*J, ...+J)).  All compute keeps
(x,y) pairs adjacent so DVE runs in 2x_1P packed-fp16 mode:
  awh[j,2] = (512*vw)*w , (512*vh)*h     (one packed STT pass)
  cxy[j,2] = 512*cx , 512*cy             (ScalarE)
  out[j,5,2] = off[j,5,2] * awh[j,1,2]   (packed TT, broadcast over 5 kps)
  out[j,5,2] += cxy[j,1,2]               (packed TT)
"""

import sys

import numpy as np

if "/opt/trn_rl_repo" not in sys.path:
    sys.path.insert(0, "/opt/trn_rl_repo")

import concourse.bacc as bacc
import concourse.bass as bass
import concourse.mybir as mybir
from concourse.tile import TileContext

N_CORES = 8
B, N = 32, 100000
C_FULL_IN = 20
C_FULL_OUT = 12
C_IN = 16   # channels 2:18 of the original layout
C_OUT = 10  # kp channels only
B_PER_CORE = B // N_CORES
ROWS = B_PER_CORE * N  # 400000 rows per core
P = 128
SCALE = 512.0
F16 = mybir.dt.float16


# Per-tile boxes-per-partition. Small first tiles start compute early
# (short pipeline fill); small last tile shortens the store tail.
# sum(J_LIST) * P == ROWS.
J_LIST = [125, 250, 450, 450, 450, 450, 400, 250, 150, 75, 50, 25]


def build_nc(rows=ROWS, j_list=None, in_bufs=6, out_bufs=5):
    """Per-core Bass program: [rows, 16] f16 -> [rows, 10] f16 kp decode."""
    if j_list is None:
        j_list = J_LIST
    assert sum(j_list) * P == rows, (sum(j_list) * P, rows)
    mult = mybir.AluOpType.mult

    nc = bacc.Bacc()
    x = nc.dram_tensor("x", [rows, C_IN], F16, kind="ExternalInput")
    y = nc.dram_tensor("out", [rows, C_OUT], F16, kind="ExternalOutput")

    with TileContext(nc) as tc:
        with (
            tc.tile_pool(name="ip", bufs=in_bufs) as ip,
            tc.tile_pool(name="op", bufs=out_bufs) as op,
            tc.tile_pool(name="tmp", bufs=3) as tp,
        ):
            r0 = 0
            for j in j_list:
                tile_rows = P * j
                xin = x[r0 : r0 + tile_rows, :].rearrange("(p j) c -> p (j c)", p=P)
                xt = ip.tile([P, j * C_IN], F16, tag="in")
                nc.sync.dma_start(out=xt[:], in_=xin)
                xv = xt[:].rearrange("p (j c) -> p j c", c=C_IN)

                ot = op.tile([P, j * C_OUT], F16, tag="out")

                # awh[j,2] = (vw,vh)*(w,h) — packed pairs. On GpSimd: it is
                # otherwise idle and 2j elems/tile is cheap even on Q7,
                # keeping DVE free for the two big 10j passes. (STT lowers
                # to TensorScalarPtr which Pool lacks, so plain TT here and
                # the *512 goes to ScalarE below. DVE STT runs at 1x — only
                # TT gets the packed-fp16 2x mode — so both 10j passes stay
                # TT on DVE.)
                awhr = tp.tile([P, j * 2], F16, tag="awhr")
                nc.gpsimd.tensor_mul(
                    out=awhr[:].rearrange("p (j c) -> p j c", c=2),
                    in0=xv[:, :, 14:16], in1=xv[:, :, 12:14],
                )
                # awh = 512*awhr ; cxy = 512*(cx,cy) — both on ScalarE
                awh = tp.tile([P, j * 2], F16, tag="awh")
                nc.scalar.mul(out=awh[:], in_=awhr[:], mul=SCALE)
                cxy = tp.tile([P, j * 2], F16, tag="cxy")
                nc.scalar.mul(
                    out=cxy[:].rearrange("p (j c) -> p j c", c=2),
                    in_=xv[:, :, 10:12], mul=SCALE,
                )

                off4 = xv[:, :, 0:10].rearrange("p j (k two) -> p j k two", two=2)
                ov4 = ot[:].rearrange("p (j k two) -> p j k two", k=5, two=2)
                awh4 = (
                    awh[:].rearrange("p (j two) -> p j two", two=2)
                    .unsqueeze(2).broadcast_to((P, j, 5, 2))
                )
                cxy4 = (
                    cxy[:].rearrange("p (j two) -> p j two", two=2)
                    .unsqueeze(2).broadcast_to((P, j, 5, 2))
                )

                nc.vector.tensor_mul(out=ov4, in0=off4, in1=awh4)
                nc.vector.tensor_add(out=ov4, in0=ov4, in1=cxy4)

                yout = y[r0 : r0 + tile_rows, :].rearrange("(p j) c -> p (j c)", p=P)
                nc.scalar.dma_start(out=yout, in_=ot[:])
                r0 += tile_rows

    nc.finalize()
    return nc


_NC_CACHE = {}


def _get_nc():
    if "nc" not in _NC_CACHE:
        _NC_CACHE["nc"] = build_nc()
    return _NC_CACHE["nc"]


def kernel(y_pred: np.ndarray) -> np.ndarray:
    from concourse.bass_utils import run_bass_kernel_spmd

    y_pred = np.asarray(y_pred, dtype=np.float32)
    assert y_pred.shape == (B, N, C_FULL_IN), y_pred.shape

    x16 = y_pred[..., 2:18].astype(np.float16)  # (B, N, 16) contiguous
    shards = x16.reshape(N_CORES, ROWS, C_IN)

    nc = _get_nc()
    in_maps = [{"x": shards[c]} for c in range(N_CORES)]
    res = run_bass_kernel_spmd(nc, in_maps, list(range(N_CORES)))
    kp = np.stack([res.results[c]["out"] for c in range(N_CORES)])

    out = np.empty((B, N, C_FULL_OUT), dtype=np.float32)
    out[..., 0:2] = y_pred[..., 0:2]  # exact passthrough
    out[..., 2:12] = kp.reshape(B, N, C_OUT).astype(np.float32)
    return out


# revision 9
# speedup vs baseline: 1.1399x; 1.1399x over previous
"""DecodeDetections keypoint-decode kernel for Trainium2 (8 NeuronCores).

Computation per box (original 20 input channels -> 12 output channels):
  out[0:2]    = in[0:2]                                       (class scores)
  out[2+2k]   = (in[2+2k] * in[16] * in[14] + in[12]) * 512   k=0..4  (kp x)
  out[3+2k]   = (in[3+2k] * in[17] * in[15] + in[13]) * 512   k=0..4  (kp y)
Channels 18,19 are unused; out[0:2] is an exact passthrough of in[0:2].

The kernel is DMA-bound (16 SDMA engines/core @ ~26 GB/s each), so the
device moves only the bytes that feed real math, in fp16:
  - host slices channels 2:18 (16ch) to fp16; the *512 NORMALIZE scale is
    folded into the offset/cx/cy channels on host (exact in fp16: pure
    exponent shift, |512*x| < 3100 << 65504)
  - device computes the 10 kp channels in fp16
  - host copies class channels from the f32 input directly (exact) and
    upcasts kp to f32.  Output absmax ~17k fits fp16; rel-err ~1e-3.

Device math per box, with off' = 512*off, cxy' = 512*(cx,cy) from host:
  awhr[j,2] = (vw,vh)*(w,h)              (GpSimd TT; DVE TT on small tiles)
  out[j,5,2] = off'[j,5,2] * awhr[j,1,2] (DVE TT, packed-fp16 2x mode)
  out[j,5,2] += cxy'[j,1,2]              (DVE TT, packed-fp16 2x mode)
All compute keeps (x,y) pairs adjacent so DVE hits 2x_1P; DVE STT and
broadcast-less variants were measured slower.  Batch axis split 4-per-core;
rows tiled partition-major (tile t, partition p holds rows
[t*128*J + p*J, ...+J)) so every DMA line is one long contiguous HBM run.
"""

import sys

import numpy as np

if "/opt/trn_rl_repo" not in sys.path:
    sys.path.insert(0, "/opt/trn_rl_repo")

import concourse.bacc as bacc
import concourse.bass as bass
import concourse.mybir as mybir
from concourse.tile import TileContext

N_CORES = 8
B, N = 32, 100000
C_FULL_IN = 20
C_FULL_OUT = 12
C_IN = 16   # channels 2:18 of the original layout
C_OUT = 10  # kp channels only
B_PER_CORE = B // N_CORES
ROWS = B_PER_CORE * N  # 400000 rows per core
P = 128
SCALE = 512.0
F16 = mybir.dt.float16


# Per-tile boxes-per-partition. Small first tile starts the pipeline fast;
# small last tiles keep the drain (last input -> last compute -> last store)
# short.  sum(J_LIST) * P == ROWS.
J_LIST = [125, 250, 450, 450, 450, 450, 450, 300, 125, 75]


def build_nc(rows=ROWS, j_list=None, in_bufs=5, out_bufs=4):
    """Per-core Bass program: [rows, 16] f16 -> [rows, 10] f16 kp decode."""
    if j_list is None:
        j_list = J_LIST
    assert sum(j_list) * P == rows, (sum(j_list) * P, rows)

    nc = bacc.Bacc()
    x = nc.dram_tensor("x", [rows, C_IN], F16, kind="ExternalInput")
    y = nc.dram_tensor("out", [rows, C_OUT], F16, kind="ExternalOutput")

    with TileContext(nc) as tc:
        with (
            tc.tile_pool(name="ip", bufs=in_bufs) as ip,
            tc.tile_pool(name="op", bufs=out_bufs) as op,
            tc.tile_pool(name="tmp", bufs=3) as tp,
        ):
            r0 = 0
            for j in j_list:
                tile_rows = P * j
                xin = x[r0 : r0 + tile_rows, :].rearrange("(p j) c -> p (j c)", p=P)
                xt = ip.tile([P, j * C_IN], F16, tag="in")
                nc.sync.dma_start(out=xt[:], in_=xin)
                xv = xt[:].rearrange("p (j c) -> p j c", c=C_IN)

                ot = op.tile([P, j * C_OUT], F16, tag="out")

                # awhr[j,2] = (vw,vh)*(w,h) — packed pairs.  Big tiles go to
                # the otherwise-idle GpSimd to keep DVE on the two 10j
                # passes; small (tail) tiles stay on DVE where the TT is
                # ~0.1us, keeping the drain chain latency minimal.
                awhr = tp.tile([P, j * 2], F16, tag="awhr")
                eng = nc.gpsimd if j > 125 else nc.vector
                eng.tensor_mul(
                    out=awhr[:].rearrange("p (j c) -> p j c", c=2),
                    in0=xv[:, :, 14:16], in1=xv[:, :, 12:14],
                )

                off4 = xv[:, :, 0:10].rearrange("p j (k two) -> p j k two", two=2)
                ov4 = ot[:].rearrange("p (j k two) -> p j k two", k=5, two=2)
                awh4 = (
                    awhr[:].rearrange("p (j two) -> p j two", two=2)
                    .unsqueeze(2).broadcast_to((P, j, 5, 2))
                )
                cxy4 = (
                    xv[:, :, 10:12]
                    .unsqueeze(2).broadcast_to((P, j, 5, 2))
                )

                nc.vector.tensor_mul(out=ov4, in0=off4, in1=awh4)
                nc.vector.tensor_add(out=ov4, in0=ov4, in1=cxy4)

                yout = y[r0 : r0 + tile_rows, :].rearrange("(p j) c -> p (j c)", p=P)
                nc.scalar.dma_start(out=yout, in_=ot[:])
                r0 += tile_rows

    nc.finalize()
    return nc


_NC_CACHE = {}


def _get_nc():
    if "nc" not in _NC_CACHE:
        _NC_CACHE["nc"] = build_nc()
    return _NC_CACHE["nc"]


def _prep_input(y_pred: np.ndarray) -> np.ndarray:
    """Slice channels 2:18, fold the *512 into offsets/cx/cy, cast fp16."""
    x16 = np.empty((B, N, C_IN), dtype=np.float16)
    x16[..., 0:12] = (y_pred[..., 2:14] * SCALE).astype(np.float16)
    x16[..., 12:16] = y_pred[..., 14:18].astype(np.float16)
    return x16


def kernel(y_pred: np.ndarray) -> np.ndarray:
    from concourse.bass_utils import run_bass_kernel_spmd

    y_pred = np.asarray(y_pred, dtype=np.float32)
    assert y_pred.shape == (B, N, C_FULL_IN), y_pred.shape

    shards = _prep_input(y_pred).reshape(N_CORES, ROWS, C_IN)

    nc = _get_nc()
    in_maps = [{"x": shards[c]} for c in range(N_CORES)]
    res = run_bass_kernel_spmd(nc, in_maps, list(range(N_CORES)))
    kp = np.stack([res.results[c]["out"] for c in range(N_CORES)])

    out = np.empty((B, N, C_FULL_OUT), dtype=np.float32)
    out[..., 0:2] = y_pred[..., 0:2]  # exact passthrough
    out[..., 2:12] = kp.reshape(B, N, C_OUT).astype(np.float32)
    return out
